# revision 10
# baseline (speedup 1.0000x reference)
"""AFT (attention-free transformer) full-sequence attention kernel for
Trainium2, data-parallel over batch across 8 NeuronCores.

Math per batch element b (one core each, B == n_cores == 8):
    proj = x @ w_attn ; q, k, v = split(proj)
    maxk = rowmax(k); ke = exp(k - maxk); kv = ke * v
    EB[i,j] = exp(pos_bias[i,j]) * (j <= i)      (maxb row-factor cancels in num/den)
    num = EB @ kv ; den = EB @ ke
    y = sigmoid(q) * num / den ; out = y @ w_proj

Device layout choices (all transposes done host-side, zero on-device transposes):
    xT  = x[b].T            [D, T]  -> lhsT tiles for k/v, rhs for qT
    qT, num^T, den^T, y^T computed in [D, T] layout so w_proj matmul needs no
    transpose; EB^T (from host-transposed pos_bias.T) is the moving operand.
    Lower-triangular structure of EB skips ~half the j-blocks.
"""

import numpy as np
import ml_dtypes

import concourse.bass as bass
import concourse.mybir as mybir
import concourse.tile as tile
from concourse import bacc
from concourse.bass import ts, ds
from concourse.bass_utils import run_bass_kernel_spmd

F32 = mybir.dt.float32
F32R = mybir.dt.float32r
BF16 = mybir.dt.bfloat16
X = mybir.AxisListType.X
MUL = mybir.AluOpType.mult
MIN = mybir.AluOpType.min

P = 128
B, T, D = 8, 2048, 1024
NDC = D // P          # 8 contraction chunks over D
NTB = T // P          # 16 t-chunks of 128
NIB = T // 512        # 4 i-blocks of 512

# dense-matmul dtype: float32r runs the PE at bf16 speed (1 cycle/row for
# moving dim >= 256) vs 4 cycles/row for plain float32. Walrus requires every
# producer of an fp32r-matmul input to emit dtype float32r, so the dense
# operands are declared float32r end-to-end (bit-identical storage to f32).
MMDT = F32R


def build_nc():
    nc = bacc.Bacc("TRN2")

    xT = nc.declare_dram_parameter("xT", [D, T], MMDT, isOutput=False)
    wq = nc.declare_dram_parameter("wq", [D, D], MMDT, isOutput=False)
    wk = nc.declare_dram_parameter("wk", [D, D], MMDT, isOutput=False)
    wv = nc.declare_dram_parameter("wv", [D, D], MMDT, isOutput=False)
    wp = nc.declare_dram_parameter("wp", [D, D], MMDT, isOutput=False)
    pbT = nc.declare_dram_parameter("pbT", [T, T], F32, isOutput=False)
    out = nc.declare_dram_parameter("out", [T, D], F32, isOutput=True)

    with tile.TileContext(nc) as tc:
        _emit(nc, tc, xT, wq, wk, wv, wp, pbT, out)
    nc.compile()
    return nc


def _emit(nc, tc, xT, wq, wk, wv, wp, pbT, out):
    import contextlib

    ctx = contextlib.ExitStack()
    with ctx:
        singles = ctx.enter_context(tc.tile_pool(name="singles", bufs=1))
        xtp = ctx.enter_context(tc.tile_pool(name="xtp", bufs=2))
        ebtp = ctx.enter_context(tc.tile_pool(name="ebtp", bufs=16))
        pbp = ctx.enter_context(tc.tile_pool(name="pbp", bufs=2))
        outp = ctx.enter_context(tc.tile_pool(name="outp", bufs=2))
        smallp = ctx.enter_context(tc.tile_pool(name="smallp", bufs=4))
        drp = ctx.enter_context(tc.tile_pool(name="drp", bufs=2))
        psump = ctx.enter_context(tc.tile_pool(name="psump", bufs=6, space="PSUM"))

        xT_r = xT[:].rearrange("(dc p) t -> p dc t", p=P)

        # kv in free cols [0, 1024), ke in [1024, 2048); chunk mc of kv is
        # cols ts(mc,128), chunk mc of ke is cols ts(mc+8,128).
        kvke = singles.tile([P, NTB, 2 * D], BF16, tag="kvke")
        sq = singles.tile([P, NDC, T], BF16, tag="sq")

        # ---------------- Phase A1: k -> ke = exp(k - rowmax(k)) -------------
        wk_t = singles.tile([P, NDC, D], MMDT, tag="wslot")
        nc.sync.dma_start(out=wk_t[:], in_=wk[:].rearrange("(dc p) f -> p dc f", p=P))

        for tblk in range(4):
            xt_t = xtp.tile([P, NDC, 512], MMDT, tag="xt")
            nc.sync.dma_start(out=xt_t[:], in_=xT_r[:, :, ts(tblk, 512)])
            for sub in range(4):
                tb = tblk * 4 + sub
                ps0 = psump.tile([P, 512], F32, tag="ps")
                ps1 = psump.tile([P, 512], F32, tag="ps")
                for half, ps in ((0, ps0), (1, ps1)):
                    for dc in range(NDC):
                        nc.tensor.matmul(
                            ps[:],
                            (xt_t[:, dc, ts(sub, P)]),
                            (wk_t[:, dc, ts(half, 512)]),
                            start=(dc == 0),
                            stop=(dc == NDC - 1),
                        )
                m0 = smallp.tile([P, 1], F32, tag="m0")
                m1 = smallp.tile([P, 1], F32, tag="m1")
                nc.vector.reduce_max(m0[:], ps0[:], axis=X, negate=True)
                nc.vector.reduce_max(m1[:], ps1[:], axis=X, negate=True)
                nm = smallp.tile([P, 1], F32, tag="nm")
                nc.vector.tensor_tensor(nm[:], m0[:], m1[:], op=MIN)
                for half, ps in ((0, ps0), (1, ps1)):
                    nc.scalar.activation(
                        out=kvke[:, tb, ds(D + half * 512, 512)],
                        in_=ps[:],
                        func=mybir.ActivationFunctionType.Exp,
                        bias=nm[:],
                    )

        # ---------------- Phase A2: v -> kv = ke * v -------------------------
        wv_t = singles.tile([P, NDC, D], MMDT, tag="wslot")
        nc.sync.dma_start(out=wv_t[:], in_=wv[:].rearrange("(dc p) f -> p dc f", p=P))

        for tblk in range(4):
            xt_t = xtp.tile([P, NDC, 512], MMDT, tag="xt")
            nc.sync.dma_start(out=xt_t[:], in_=xT_r[:, :, ts(tblk, 512)])
            for sub in range(4):
                tb = tblk * 4 + sub
                for half in range(2):
                    ps = psump.tile([P, 512], F32, tag="ps")
                    for dc in range(NDC):
                        nc.tensor.matmul(
                            ps[:],
                            (xt_t[:, dc, ts(sub, P)]),
                            (wv_t[:, dc, ts(half, 512)]),
                            start=(dc == 0),
                            stop=(dc == NDC - 1),
                        )
                    nc.vector.tensor_tensor(
                        kvke[:, tb, ds(half * 512, 512)],
                        ps[:],
                        kvke[:, tb, ds(D + half * 512, 512)],
                        op=MUL,
                    )

        # ---------------- Phase B: qT -> sigmoid(qT) -------------------------
        wq_t = singles.tile([P, NDC, D], MMDT, tag="wslot")
        nc.sync.dma_start(out=wq_t[:], in_=wq[:].rearrange("(dc p) f -> p dc f", p=P))

        for ib in range(NIB):
            xt_t = xtp.tile([P, NDC, 512], MMDT, tag="xt")
            nc.sync.dma_start(out=xt_t[:], in_=xT_r[:, :, ts(ib, 512)])
            for mc in range(NDC):
                ps = psump.tile([P, 512], F32, tag="ps")
                for dc in range(NDC):
                    nc.tensor.matmul(
                        ps[:],
                        (wq_t[:, dc, ts(mc, P)]),
                        (xt_t[:, dc, :]),
                        start=(dc == 0),
                        stop=(dc == NDC - 1),
                    )
                nc.scalar.activation(
                    out=sq[:, mc, ts(ib, 512)],
                    in_=ps[:],
                    func=mybir.ActivationFunctionType.Sigmoid,
                )

        # ---------------- Phase C: num/den, y, out ---------------------------
        wp_t = singles.tile([P, NDC, D], MMDT, tag="wslot")
        nc.sync.dma_start(out=wp_t[:], in_=wp[:].rearrange("(dc p) f -> p dc f", p=P))

        for ib in range(NIB):
            njc = 4 * (ib + 1)
            ebts = []
            for jc in range(njc):
                pb_t = pbp.tile([P, 512], F32, tag="pb")
                nc.sync.dma_start(out=pb_t[:], in_=pbT[ts(jc, P), ts(ib, 512)])
                e_t = ebtp.tile([P, 512], BF16, tag="ebt")
                nc.scalar.activation(
                    out=e_t[:], in_=pb_t[:], func=mybir.ActivationFunctionType.Exp
                )
                ebts.append(e_t)

            yt_t = singles.tile([P, NDC, 512], MMDT, tag="yt")
            for mc in range(NDC):
                psd = psump.tile([P, 512], F32, tag="ps")
                for j in range(njc):
                    nc.tensor.matmul(
                        psd[:],
                        kvke[:, j, ts(mc + NDC, P)],
                        ebts[j][:],
                        start=(j == 0),
                        stop=(j == njc - 1),
                    )
                dr = drp.tile([P, 512], F32, tag="dr")
                nc.vector.reciprocal(dr[:], psd[:])
                psn = psump.tile([P, 512], F32, tag="ps")
                for j in range(njc):
                    nc.tensor.matmul(
                        psn[:],
                        kvke[:, j, ts(mc, P)],
                        ebts[j][:],
                        start=(j == 0),
                        stop=(j == njc - 1),
                    )
                nc.vector.tensor_tensor(yt_t[:, mc, :], psn[:], dr[:], op=MUL)
                nc.vector.tensor_tensor(
                    yt_t[:, mc, :], yt_t[:, mc, :], sq[:, mc, ts(ib, 512)], op=MUL
                )

            for tc2 in range(4):
                for nb in range(2):
                    pso = psump.tile([P, 512], F32, tag="ps")
                    for dc in range(NDC):
                        nc.tensor.matmul(
                            pso[:],
                            (yt_t[:, dc, ts(tc2, P)]),
                            (wp_t[:, dc, ts(nb, 512)]),
                            start=(dc == 0),
                            stop=(dc == NDC - 1),
                        )
                    o_t = outp.tile([P, 512], F32, tag="o")
                    nc.scalar.copy(out=o_t[:], in_=pso[:])
                    nc.sync.dma_start(
                        out=out[ds(ib * 512 + tc2 * P, P), ts(nb, 512)], in_=o_t[:]
                    )


def make_in_maps(x, w_attn, w_proj, pos_bias):
    xT_all = np.ascontiguousarray(np.transpose(np.asarray(x, np.float32), (0, 2, 1)))
    w_attn = np.asarray(w_attn, np.float32)
    wq = np.ascontiguousarray(w_attn[:, :D])
    wk = np.ascontiguousarray(w_attn[:, D : 2 * D])
    wv = np.ascontiguousarray(w_attn[:, 2 * D :])
    wp = np.ascontiguousarray(np.asarray(w_proj, np.float32))
    # Transposed pos_bias with -30000 in the masked (j > i) region: the
    # device-side exp underflows those entries to exactly 0, implementing the
    # causal mask with no extra mask tensor.
    pbT = np.asarray(pos_bias, np.float32).T.copy()
    jj = np.arange(T)[:, None]
    ii = np.arange(T)[None, :]
    pbT[jj > ii] = -30000.0

    shared = dict(wq=wq, wk=wk, wv=wv, wp=wp, pbT=pbT)
    return [dict(xT=xT_all[i], **shared) for i in range(B)]


_NC_CACHE = {}


def get_nc():
    if "nc" not in _NC_CACHE:
        _NC_CACHE["nc"] = build_nc()
    return _NC_CACHE["nc"]


def kernel(x, w_attn, w_proj, pos_bias):
    nc = get_nc()
    in_maps = make_in_maps(x, w_attn, w_proj, pos_bias)
    res = run_bass_kernel_spmd(nc, in_maps, core_ids=list(range(B)))
    return np.stack([res.results[i]["out"] for i in range(B)]).astype(np.float32)
